# revision 1
# baseline (speedup 1.0000x reference)
"""Trainium2 Bass kernel for nn_CutoffModule (CBAM-style channel gate + topk gather).

Reference computation (per sample):
    avg/max spatial pooling -> shared 2-layer MLP -> sum -> sigmoid -> attn [C, D]
    per scale d: top-128 channels (sorted desc) -> gather those channels of x.

Sharding: data-parallel over N across 8 cores (4 samples/core); MLP weights
replicated. Entirely self-contained: hardcodes N=32, C=512, H=W=64, D=4, r=16.

Notes:
- sigmoid is strictly monotonic, so top_k(sigmoid(y)) == top_k(y); the kernel
  ranks pre-sigmoid logits and never materializes the sigmoid.
- topk row (d, n) lives on SBUF partition 32*d + n: engine writes must start
  at partition 0/32/64/96, and this layout lets plain DVE copies slice
  y[n, d::4] out of PSUM with legal partition offsets.
- samples are processed in two pairs so the gather+store DMA of pair 0
  overlaps the MLP+topk of pair 1.
"""

import numpy as np

import concourse.bacc as bacc
import concourse.bass as bass
import concourse.tile as tile
from concourse.tile import add_dep_helper
from concourse import mybir
from concourse.bass_utils import run_bass_kernel_spmd

# Problem constants (hardcoded per harness contract)
N_FULL = 32
C = 512
HW = 64 * 64          # 4096
D = 4                 # depth scales
BLOCK = C // D        # 128
HID = C // 16         # 32  (MLP hidden)
N_CORES = 8
NS = N_FULL // N_CORES  # 4 samples per core
P = 128               # SBUF partitions
CT = C // P           # 4 channel tiles per sample
NEG_FILL = -1e30

F32 = mybir.dt.float32
U32 = mybir.dt.uint32


def _build_program():
    nc = bacc.Bacc("TRN2", target_bir_lowering=False, debug=False)

    x_d = nc.dram_tensor("x", [NS * C, HW], F32, kind="ExternalInput").ap()
    w1_d = nc.dram_tensor("w1", [C, HID], F32, kind="ExternalInput").ap()
    b1_d = nc.dram_tensor("b1", [HID, 1], F32, kind="ExternalInput").ap()
    # w2aug = [W2; 2*b2] so layer 2 + both bias adds fold into one K=33 matmul
    w2_d = nc.dram_tensor("w2aug", [HID + 1, C * D], F32, kind="ExternalInput").ap()
    ident_d = nc.dram_tensor("ident", [P, P], F32, kind="ExternalInput").ap()
    nofs_d = nc.dram_tensor("nofs", [P, 2], F32, kind="ExternalInput").ap()
    out_d = nc.dram_tensor("out", [NS * C, HW], F32, kind="ExternalOutput").ap()

    with tile.TileContext(nc) as tc:
        with (
            tc.tile_pool(name="xin0", bufs=4) as xin0_pool,
            tc.tile_pool(name="xin1", bufs=3) as xin1_pool,
            tc.tile_pool(name="gbuf", bufs=3) as g_pool,
            tc.tile_pool(name="small", bufs=1) as sm,
            tc.tile_pool(name="psum", bufs=1, space="PSUM") as psum,
        ):
            # ---- constants / weights into SBUF (scalar ring; x loads use sync) ----
            w1_sb = sm.tile([P, CT, HID], F32)   # chunk ct = channels ct*128..+128
            nc.scalar.dma_start(
                out=w1_sb[:], in_=w1_d.rearrange("(c p) m -> p c m", p=P)
            )
            w2_sb = sm.tile([HID + 1, C * D], F32)
            nc.scalar.dma_start(out=w2_sb[:], in_=w2_d)
            b1_sb = sm.tile([HID, 1], F32)
            nc.scalar.dma_start(out=b1_sb[:], in_=b1_d)
            ident_sb = sm.tile([P, P], F32)
            nc.scalar.dma_start(out=ident_sb[:], in_=ident_d)
            nofs_sb = sm.tile([P, 2], F32)
            nc.scalar.dma_start(out=nofs_sb[:], in_=nofs_d)

            # pooling accumulators, one pair per tile so pair 0's MLP does not
            # depend on pair 1's reduces: [P, ct, {avg0,avg1,max0,max1}]
            pools = [sm.tile([P, CT, 4], F32, name=f"pools{pp}") for pp in range(2)]
            scratch = sm.tile([P, HW], F32)

            # per-pair topk tiles (rows at partition 32*d + n; rest zeroed)
            vals = [[sm.tile([P, C], F32, name=f"vals{pp}_{i}") for i in range(2)]
                    for pp in range(2)]
            for pp in range(2):
                for i in range(2):
                    nc.gpsimd.memset(vals[pp][i][:], 0.0)

            def load_and_pool(n, after=None):
                # pair 0 on the sync HWDGE ring, pair 1 on the gpsimd SWDGE ring:
                # both pairs stream in parallel, and neither ring is issued from
                # an engine with pending compute (scalar/ACT must stay clear).
                for ct in range(CT):
                    row0 = (n * CT + ct) * P
                    pool = xin0_pool if n < 2 else xin1_pool
                    xt = pool.tile([P, HW], F32, tag="xt")
                    eng = nc.sync if n < 2 else nc.gpsimd
                    eng.dma_start(out=xt[:], in_=x_d[row0 : row0 + P, :])
                    # avg pool on ScalarE: accum_out sums copy(x * 1/HW)
                    pp, i = divmod(n, 2)
                    nc.scalar.activation(
                        out=scratch[:],
                        in_=xt[:],
                        func=mybir.ActivationFunctionType.Copy,
                        scale=1.0 / HW,
                        accum_out=pools[pp][:, ct, i : i + 1],
                    )
                    nc.vector.reduce_max(
                        out=pools[pp][:, ct, 2 + i : 3 + i],
                        in_=xt[:],
                        axis=mybir.AxisListType.X,
                    )

            def mlp_pair(pp):
                """MLP for samples {2pp, 2pp+1}: psum py rows 32d+i = y[2pp+i, :]."""
                ph = psum.tile([HID, 4], F32, space="PSUM", tag="ph")
                for ct in range(CT):
                    nc.tensor.matmul(
                        out=ph[:],
                        lhsT=w1_sb[:, ct, :],
                        rhs=pools[pp][:, ct, :],
                        start=(ct == 0),
                        stop=(ct == CT - 1),
                    )
                hTa = sm.tile([HID, 2], F32, name=f"hTa{pp}")
                hTm = sm.tile([HID, 2], F32, name=f"hTm{pp}")
                nc.scalar.activation(
                    out=hTa[:], in_=ph[:, 0:2],
                    func=mybir.ActivationFunctionType.Relu, bias=b1_sb[:, :],
                )
                nc.scalar.activation(
                    out=hTm[:], in_=ph[:, 2:4],
                    func=mybir.ActivationFunctionType.Relu, bias=b1_sb[:, :],
                )
                hsum = sm.tile([HID, 2], F32, name=f"hsum{pp}")
                nc.vector.tensor_add(out=hsum[:], in0=hTa[:], in1=hTm[:])
                # augmented lhsT: rows 0-31 = hsum replicated at cols 32d+i,
                # row 32 = 1.0 (bias row of w2aug)
                hw_t = sm.tile([HID + 1, P], F32, name=f"hw{pp}")
                nc.gpsimd.memset(hw_t[:], 0.0)
                nc.vector.memset(hw_t[32:33, :], 1.0)
                for d in range(D):
                    nc.vector.tensor_copy(
                        out=hw_t[0:HID, 32 * d : 32 * d + 2], in_=hsum[:]
                    )

                py = psum.tile([P, C * D], F32, space="PSUM", tag="py")
                for s in range(C * D // 512):
                    sl = slice(s * 512, (s + 1) * 512)
                    nc.tensor.matmul(
                        out=py[:, sl], lhsT=hw_t[:], rhs=w2_sb[:, sl],
                        start=True, stop=True,
                    )
                # vals[32d+n, c] = y[n, c*D + d]
                va = vals[pp][0]
                for d in range(D):
                    nc.vector.tensor_copy(
                        out=va[32 * d : 32 * d + 2, :],
                        in_=py[32 * d : 32 * d + 2, d :: D],
                    )

            def topk_pair(pp):
                """Returns idxT tile: column 32d+n holds topk row (d, n) + n*512."""
                topk_idx = sm.tile([P, BLOCK], U32, name=f"tki{pp}")
                maxv = sm.tile([P, 8], F32, name=f"maxv{pp}")
                cur, nxt = vals[pp]
                for k in range(BLOCK // 8):
                    nc.vector.max(out=maxv[:], in_=cur[:])
                    nc.vector.max_index(
                        out=topk_idx[:, 8 * k : 8 * k + 8],
                        in_max=maxv[:],
                        in_values=cur[:],
                    )
                    if k < BLOCK // 8 - 1:
                        nc.vector.match_replace(
                            out=nxt[:], in_to_replace=maxv[:], in_values=cur[:],
                            imm_value=NEG_FILL,
                        )
                        cur, nxt = nxt, cur

                idx_f = sm.tile([P, BLOCK], F32, name=f"idxf{pp}")
                last_dve = nc.vector.tensor_copy(out=idx_f[:], in_=topk_idx[:])
                nc.vector.tensor_scalar_add(
                    idx_f[:], idx_f[:], nofs_sb[:, pp : pp + 1]
                )
                pt = psum.tile([P, P], F32, space="PSUM", tag="pt")
                nc.tensor.transpose(out=pt[:], in_=idx_f[:], identity=ident_sb[:])
                idxT = sm.tile([P, P], U32, name=f"idxT{pp}")
                nc.vector.tensor_copy(out=idxT[:], in_=pt[:])
                return idxT, last_dve

            def gather_pair(pp, idxT):
                for i, n in enumerate((2 * pp, 2 * pp + 1)):
                    for d in range(D):
                        g = g_pool.tile([P, HW], F32, tag="g")
                        nc.gpsimd.indirect_dma_start(
                            out=g[:],
                            out_offset=None,
                            in_=x_d[:, :],
                            in_offset=bass.IndirectOffsetOnAxis(
                                ap=idxT[:, 32 * d + i : 32 * d + i + 1], axis=0
                            ),
                        )
                        o0 = n * C + d * BLOCK
                        nc.sync.dma_start(out=out_d[o0 : o0 + BLOCK, :], in_=g[:])

            # emission order sets scheduler priority: pair 0 chain first, so
            # pair 1's loads/MLP/topk overlap pair 0's gather+store phase.
            for n in (0, 1):
                load_and_pool(n)
            mlp_pair(0)
            idxT0, _ = topk_pair(0)
            for n in (2, 3):
                load_and_pool(n)
            gather_pair(0, idxT0)
            mlp_pair(1)
            idxT1, _ = topk_pair(1)
            gather_pair(1, idxT1)

    nc.compile()
    return nc


_NC_CACHE = None


def _get_nc():
    global _NC_CACHE
    if _NC_CACHE is None:
        _NC_CACHE = _build_program()
    return _NC_CACHE


def _make_in_maps(x, W1, b1, W2, b2):
    x = np.ascontiguousarray(np.asarray(x, dtype=np.float32)).reshape(N_FULL, C, HW)
    W1 = np.asarray(W1, dtype=np.float32)
    b1 = np.asarray(b1, dtype=np.float32).reshape(HID, 1)
    W2 = np.asarray(W2, dtype=np.float32)
    b2 = np.asarray(b2, dtype=np.float32).reshape(1, C * D)
    w2aug = np.ascontiguousarray(np.vstack([W2, 2.0 * b2]))
    ident = np.eye(P, dtype=np.float32)
    # partition 32d + i -> topk row (d, n=2*pair+i): DRAM row base = n*512
    pidx = np.arange(P)
    nofs = np.zeros((P, 2), np.float32)
    for pp in range(2):
        nofs[:, pp] = np.where(pidx % 32 < 2, (2 * pp + pidx % 32) * C, 0)
    in_maps = []
    for core in range(N_CORES):
        shard = x[core * NS : (core + 1) * NS].reshape(NS * C, HW)
        in_maps.append(
            {
                "x": np.ascontiguousarray(shard),
                "w1": W1,
                "b1": b1,
                "w2aug": w2aug,
                "ident": ident,
                "nofs": nofs,
            }
        )
    return in_maps


def run(inputs, trace=False, **kwargs):
    """Run the SPMD kernel; returns (full_output, BassKernelResults)."""
    nc = _get_nc()
    in_maps = _make_in_maps(
        inputs["x"], inputs["W1"], inputs["b1"], inputs["W2"], inputs["b2"]
    )
    res = run_bass_kernel_spmd(
        nc, in_maps, core_ids=list(range(N_CORES)), trace=trace, **kwargs
    )
    parts = [res.results[i]["out"].reshape(NS, C, 64, 64) for i in range(N_CORES)]
    out = np.concatenate(parts, axis=0)
    return out, res


def kernel(**inputs) -> np.ndarray:
    out, _ = run(inputs)
    return out

